# revision 61
# baseline (speedup 1.0000x reference)
"""Trainium2 Bass kernel for batch-8 multi-head attention (B=8, N=1024, C=768, H=12).

Distribution: pure data parallelism — batch element i runs entirely on core i
(weights replicated, zero collectives, full inputs sharded on host).

Per-core pipeline (bf16 matmuls, fp32 PSUM accumulation, 512-wide halves):
  xT[k, t]      PE transposes of the DMA'd x tiles (k-outer for early chunks)
  qT/kT         W_qkv chunk (stationary) x xT (moving), evacuated into
                zero-padded [128, N] per-head blocks: rows 64:127 are zeros so
                scores stream a full 128-wide contraction (64-wide moving
                operands run at half rate on the PE — measured 427 vs 216 ns)
  v[t, c]       xT chunk (stationary) x W_v (moving), natural layout, stored
                as H blocks of [v_h(64) | ones(1)] per s-tile
  ST[s, t]      kT slice (stationary) x padded qT (moving)       (scores^T)
  expST         ACT exp(SCALE * ST) PSUM->SBUF bf16; the 96 exps are the
                ~102us ACT floor that paces the attention stream
  y65[t, d|cs]  natural-layout AV: expST slice (stationary) x [v|1] (moving);
                col 64 = softmax denominator as a per-partition column ->
                cheap [128,1] reciprocal + fused tensor_scalar normalize
  yT            PE transposes of y_nat back to [c, t] for the projection
  z[t, c]       yT slice (stationary) x W_proj (moving) + bias broadcast

Head loop is software-pipelined (scores/exp of head h overlap AV/norm of head
h-1); heads 0-1 are emitted inside the qkv phase so the exp stream starts
while qkv is still running. DMAs are split across both HWDGE queues.
"""
import numpy as np

import concourse.bacc as bacc
import concourse.bass as bass
import concourse.tile as tile
import concourse.mybir as mybir
from concourse import masks
from concourse.bass_utils import run_bass_kernel_spmd

F32 = mybir.dt.float32
BF16 = mybir.dt.bfloat16

B, N, C = 8, 1024, 768
H, D = 12, 64
SCALE = float(D) ** -0.5
N_CORES = 8
KT = C // 128            # 6 contraction chunks of 128
TT = N // 128            # 8 token tiles of 128
ST = N // 128            # 8 key tiles of 128
EXP_FN = mybir.ActivationFunctionType.Exp


def build_nc():
    nc = bacc.Bacc("TRN2", target_bir_lowering=False, debug=False,
                   num_devices=N_CORES)
    x_ext = nc.dram_tensor("x", [N, C], F32, kind="ExternalInput")
    wqkv_ext = nc.dram_tensor("W_qkv", [C, 3 * C], F32, kind="ExternalInput")
    wproj_ext = nc.dram_tensor("W_proj", [C, C], F32, kind="ExternalInput")
    bproj_ext = nc.dram_tensor("b_proj", [C], F32, kind="ExternalInput")
    out_ext = nc.dram_tensor("out", [N, C], F32, kind="ExternalOutput")

    with tile.TileContext(nc) as tc:
        with (
            tc.tile_pool(name="const", bufs=1) as constp,
            tc.tile_pool(name="wq", bufs=1) as wqp,
            tc.tile_pool(name="xt", bufs=1) as xtp,
            tc.tile_pool(name="qk", bufs=1) as qkp,
            tc.tile_pool(name="vp", bufs=1) as vp,
            tc.tile_pool(name="yt", bufs=1) as ytp,
            tc.tile_pool(name="yn", bufs=1) as ynp,
            tc.tile_pool(name="recip", bufs=1) as recipp,
            tc.tile_pool(name="psum", bufs=3, space="PSUM") as psum,
        ):
            # ---- constants ----
            ident = constp.tile([128, 128], BF16)
            masks.make_identity(nc, ident[:])
            ident_f = constp.tile([128, 128], F32)
            masks.make_identity(nc, ident_f[:])
            ones_bf = constp.tile([128, 128], BF16)
            nc.gpsimd.memset(ones_bf[:], 1.0)
            b_sb = constp.tile([1, C], F32)
            nc.sync.dma_start(b_sb[:], bproj_ext[:].rearrange("(a c) -> a c", a=1))
            b_bf = constp.tile([1, C], BF16)
            nc.vector.tensor_copy(b_bf[:], b_sb[:])
            b_bcast = constp.tile([128, C], BF16)

            # ---- persistent tensors ----
            xt_bf = xtp.tile([128, KT * N], BF16)          # xT: chunk k at cols [k*N, (k+1)*N)
            wq_bf = wqp.tile([128, KT * 3 * C], BF16)      # W_qkv chunk k at cols [k*3C, ...)
            wp_bf = wqp.tile([128, KT * C], BF16)          # W_proj chunk k at cols [k*C, ...)
            # qT,kT: 24 per-head blocks [128, N]; rows 64:128 are zeros so the
            # scores matmul streams a full-width 128-contraction (64-wide
            # moving operands run at half rate on the PE)
            qk_pad = qkp.tile([128, 24 * N], BF16)
            v65 = vp.tile([128, ST * H * 65], BF16)        # per s-tile: H blocks of [v_h(64)|1]
            y_nat = ynp.tile([128, TT * C], BF16)          # y natural: t-tile t at cols [t*C, ...)
            yt_bf = ytp.tile([128, KT * N], BF16)          # yT: chunk c at cols [c*N, ...)

            def halves(width):
                out = []
                off = 0
                while off < width:
                    w = min(512, width - off)
                    out.append((off, w))
                    off += w
                return out

            stage_ctx = tc.tile_pool(name="wstage", bufs=1)
            wstage = stage_ctx.__enter__()
            stage_ctx2 = tc.tile_pool(name="xstage", bufs=2)
            xstage = stage_ctx2.__enter__()
            # ---- phase 1+2: load x (sync queue) and W (scalar DMA queue) ----
            with nc.named_scope("xload"):
                x_tiles = []
                for t in range(TT):
                    x_f = xstage.tile([128, C], F32, tag="xf", bufs=TT)
                    nc.sync.dma_start(x_f[:], x_ext[t * 128:(t + 1) * 128, :])
                    x_tiles.append(x_f)
                # k-outer so xT chunk k completes as early as possible
                for k in range(KT):
                    for t in range(TT):
                        tag = "ps" if k % 2 == 0 else "ys"
                        tp_ps = psum.tile([128, 128], F32, tag=tag, bufs=3 if tag == "ps" else 2)
                        nc.tensor.transpose(tp_ps[:], x_tiles[t][:, k * 128:(k + 1) * 128], ident_f[:])
                        dst = xt_bf[:, k * N + t * 128: k * N + (t + 1) * 128]
                        if k % 2 == 0:
                            nc.vector.tensor_copy(dst, tp_ps[:])
                        else:
                            nc.scalar.copy(dst, tp_ps[:])

            with nc.named_scope("wload"):
                for k in range(KT):
                    w_f = wstage.tile([128, 3 * C], F32, tag=f"wf{k % 4}")
                    dma_eng = nc.scalar if k < 4 else nc.sync
                    dma_eng.dma_start(w_f[:], wqkv_ext[k * 128:(k + 1) * 128, :])
                    # convert in thirds (q|k|v col groups) for finer scheduling
                    for g in range(3):
                        nc.vector.tensor_copy(
                            wq_bf[:, k * 3 * C + g * C: k * 3 * C + (g + 1) * C],
                            w_f[:, g * C:(g + 1) * C])

            # zero the pad rows of the qk blocks once (gpsimd is idle here)
            Q4 = 24 * N // 4
            for _g in range(4):
                nc.gpsimd.memset(qk_pad[64:128, _g * Q4:(_g + 1) * Q4], 0.0)

            with nc.named_scope("wproj_load"):
                for k in range(KT):
                    w_f = wstage.tile([128, 3 * C], F32, tag=f"wf{k % 4}")
                    nc.sync.dma_start(w_f[:, 0:C], wproj_ext[k * 128:(k + 1) * 128, :])
                    if k % 2 == 0:
                        nc.gpsimd.tensor_copy(wp_bf[:, k * C:(k + 1) * C], w_f[:, 0:C])
                    else:
                        nc.vector.tensor_copy(wp_bf[:, k * C:(k + 1) * C], w_f[:, 0:C])

                # b broadcast to 128 partitions via bf16 matmul
                b_ps = psum.tile([128, C], F32, tag="ps")
                for off, w in halves(C):
                    nc.tensor.matmul(b_ps[:, off:off + w], ones_bf[0:1, 0:128],
                                     b_bf[0:1, off:off + w], start=True, stop=True)
                nc.scalar.copy(b_bcast[:], b_ps[:])

            # ---- phase 4: qT,kT (head-pair order) with v tiles interleaved ----
            def emit_v_tile(t):
                v_ps = psum.tile([128, C], F32, tag="ps")
                for k in range(KT):
                    lhsT = xt_bf[:, k * N + t * 128: k * N + (t + 1) * 128]
                    for off, w in halves(C):
                        nc.tensor.matmul(v_ps[:, off:off + w], lhsT,
                                         wq_bf[:, k * 3 * C + 2 * C + off: k * 3 * C + 2 * C + off + w],
                                         start=(k == 0), stop=(k == KT - 1))
                base = t * H * 65
                v_view = v65[:, base: base + H * 65].rearrange("p (h w) -> p h w", w=65)
                nc.scalar.copy(v_view[:, :, 0:64],
                               v_ps[:].rearrange("p (h d) -> p h d", d=64))
                nc.gpsimd.memset(v_view[:, :, 64:65], 1.0)

            def emit_qk_col(co, evac_dve=False):
                # co 0..5 = q pair (heads 2co, 2co+1); co 6..11 = k pair
                qk_ps = psum.tile([128, N], F32, tag="ps")
                for k in range(KT):
                    lhsT = wq_bf[:, k * 3 * C + co * 128: k * 3 * C + (co + 1) * 128]
                    for off, w in halves(N):
                        nc.tensor.matmul(qk_ps[:, off:off + w], lhsT,
                                         xt_bf[:, k * N + off: k * N + off + w],
                                         start=(k == 0), stop=(k == KT - 1))
                if co < KT:
                    blk0, blk1 = 2 * co, 2 * co + 1
                else:
                    blk0, blk1 = 12 + 2 * (co - KT), 12 + 2 * (co - KT) + 1
                ev = nc.scalar.copy if co in (0, 1, KT, KT + 1) else nc.vector.tensor_copy
                ev(qk_pad[0:64, blk0 * N:(blk0 + 1) * N], qk_ps[0:64, :])
                ev(qk_pad[0:64, blk1 * N:(blk1 + 1) * N], qk_ps[64:128, :])

            _qs = nc.enter_named_scope("qkv", False)

            stage_ctx2.__exit__(None, None, None)
            stage_ctx.__exit__(None, None, None)
            exp_ctx = tc.tile_pool(name="exp", bufs=16)
            expp = exp_ctx.__enter__()
            z_ctx = tc.tile_pool(name="z", bufs=2)
            zp = z_ctx.__enter__()
            # ---- phase 5: attention, software-pipelined across heads ----
            e_tiles = {}

            def emit_scores_exp(h):
                q_ap = qk_pad[:, h * N:(h + 1) * N]
                k_ap = qk_pad[:, (12 + h) * N:(12 + h + 1) * N]
                tiles = []
                for s in range(ST):
                    s_ps = psum.tile([128, N], F32, tag="ps")
                    for off, w in halves(N):
                        nc.tensor.matmul(s_ps[:, off:off + w],
                                         k_ap[:, s * 128:(s + 1) * 128],
                                         q_ap[:, off:off + w],
                                         start=True, stop=True)
                    e_t = expp.tile([128, N], BF16, tag="exp")
                    nc.scalar.activation(e_t[:], s_ps[:], EXP_FN, bias=0.0, scale=SCALE)
                    tiles.append(e_t)
                e_tiles[h] = tiles

            def emit_av_norm(h):
                # natural-layout AV: out[t_tile][t, d|colsum]; colsum is a
                # per-partition column -> cheap reciprocal + tensor_scalar
                tiles = e_tiles.pop(h)
                for t in range(TT):
                    y_ps = psum.tile([128, 65], F32, tag="ys", bufs=2)
                    for s in range(ST):
                        lhsT = tiles[s][:, t * 128:(t + 1) * 128]
                        rhs = v65[:, s * H * 65 + h * 65: s * H * 65 + (h + 1) * 65]
                        nc.tensor.matmul(y_ps[:, 0:65], lhsT, rhs,
                                         start=(s == 0), stop=(s == ST - 1))
                    recip = recipp.tile([128, 1], F32, tag="recip", bufs=4)
                    nc.vector.reciprocal(recip[:, 0:1], y_ps[:, 64:65])
                    dst = y_nat[:, t * C + h * 64: t * C + (h + 1) * 64]
                    nc.vector.tensor_scalar_mul(dst, y_ps[:, 0:64], recip[:, 0:1])

            def emit_ytrans(i):
                # transpose y_nat c-chunk i (heads 2i, 2i+1) into yt_bf
                for t in range(TT):
                    tp_ps = psum.tile([128, 128], BF16, tag="ys", bufs=2)
                    nc.tensor.transpose(tp_ps[:],
                                        y_nat[:, t * C + i * 128: t * C + (i + 1) * 128],
                                        ident[:])
                    dst = yt_bf[:, i * N + t * 128: i * N + (t + 1) * 128]
                    if t % 2 == 0:
                        nc.vector.tensor_copy(dst, tp_ps[:])
                    else:
                        nc.scalar.copy(dst, tp_ps[:])

            emit_qk_col(0)
            emit_qk_col(KT)
            emit_qk_col(1)
            emit_qk_col(KT + 1)
            for t in range(TT):
                emit_v_tile(t)
            emit_scores_exp(0)
            emit_scores_exp(1)
            for i in range(2, KT):
                emit_qk_col(i)
                emit_qk_col(KT + i)
            nc.leave_named_scope("qkv", _qs[0], False)

            _as = nc.enter_named_scope("attn", False)
            emit_av_norm(0)
            for h in range(2, H):
                emit_scores_exp(h)
                emit_av_norm(h - 1)
                if (h - 1) % 2 == 1:
                    emit_ytrans((h - 1) // 2)
            emit_av_norm(H - 1)
            emit_ytrans((H - 1) // 2)
            nc.leave_named_scope("attn", _as[0], False)

            # ---- phase 6: out = yT^T @ W_proj + b ----
            _ps_ = nc.enter_named_scope("proj", False)
            for t in range(TT):
                z_ps = psum.tile([128, C], F32, tag="ps")
                for k in range(KT):
                    lhsT = yt_bf[:, k * N + t * 128: k * N + (t + 1) * 128]
                    for off, w in halves(C):
                        nc.tensor.matmul(z_ps[:, off:off + w], lhsT,
                                         wp_bf[:, k * C + off: k * C + off + w],
                                         start=(k == 0), stop=(k == KT - 1))
                z_sb = zp.tile([128, C], F32, tag="z")
                nc.vector.tensor_add(z_sb[:], z_ps[:], b_bcast[:])
                nc.sync.dma_start(out_ext[t * 128:(t + 1) * 128, :], z_sb[:])
            nc.leave_named_scope("proj", _ps_[0], False)

            z_ctx.__exit__(None, None, None)
            exp_ctx.__exit__(None, None, None)

    nc.finalize()
    return nc


_NC = None


def _get_nc():
    global _NC
    if _NC is None:
        _NC = build_nc()
    return _NC


def _run(x, W_qkv, W_proj, b_proj, trace=False):
    nc = _get_nc()
    W_qkv = np.ascontiguousarray(W_qkv, dtype=np.float32)
    W_proj = np.ascontiguousarray(W_proj, dtype=np.float32)
    b_proj = np.ascontiguousarray(b_proj, dtype=np.float32)
    in_maps = [
        {
            "x": np.ascontiguousarray(x[i], dtype=np.float32),
            "W_qkv": W_qkv,
            "W_proj": W_proj,
            "b_proj": b_proj,
        }
        for i in range(N_CORES)
    ]
    res = run_bass_kernel_spmd(nc, in_maps, core_ids=list(range(N_CORES)),
                               trace=trace)
    out = np.stack([res.results[i]["out"] for i in range(N_CORES)], axis=0)
    return out.astype(np.float32), res


def kernel(x, W_qkv, W_proj, b_proj):
    out, _ = _run(x, W_qkv, W_proj, b_proj, trace=False)
    return out
